# revision 2
# baseline (speedup 1.0000x reference)
"""Trainium2 Bass kernel for nn_Decoder_30949534335472 (hazard-MLP decoder).

Math (per token, H=512):
    h2  = h @ W_h1.T + b_h1
    res = relu(h @ W_r.T + b_r)
    a0  = tanh(h2);            z0 = tanh(a0 @ W1p.T + b1p)
    a   = tanh(h2 + wt*t);     z  = tanh(a @ W1p.T + b1p)
    hazard    = W2p·(z - z0) + res*t          (b2p cancels)
    intensity = W2p·[(1-z^2) ⊙ (W1p @ ((1-a^2) ⊙ wt))] + res + 1e-6
where wt/W1p/b1p/W2p are the relu-masked hazard params (jax.grad replaced
by its closed form).

Sharding: pure data parallel, batch axis 16 -> 8 cores x 2.
Layout on device: feature-major ([feature 128-chunk, token] tiles) so every
matmul contracts on the partition dim; h is cast to bf16 on host and loaded
transposed via DMA-transpose.
"""

import sys

sys.path.insert(0, "/opt/trn_rl_repo")

from contextlib import ExitStack

import ml_dtypes
import numpy as np

import concourse.bass as bass
import concourse.tile as tile
from concourse import bacc, mybir
from concourse import bass_utils

F32 = mybir.dt.float32
BF16 = mybir.dt.bfloat16
AF = mybir.ActivationFunctionType
ALU = mybir.AluOpType

B, S, H = 16, 2048, 512
NCORES = 8
NTOK = (B // NCORES) * S  # 4096 tokens per core
T = 512                   # tokens per tile
NTILE = NTOK // T         # 8
KC = H // 128             # 4 feature chunks
MINPOS = 1e-6

_NC_CACHE = None


def _build_body(ctx: ExitStack, tc: "tile.TileContext", io: dict):
    nc = tc.nc
    hb, td = io["hb"], io["t"]
    out_i, out_h = io["intensity"], io["hazard"]

    wpool = ctx.enter_context(tc.tile_pool(name="weights", bufs=1))

    whT_s = wpool.tile([128, KC, H], BF16)
    w1T_s = wpool.tile([128, KC, H], BF16)
    for k in range(KC):
        nc.sync.dma_start(whT_s[:, k], io["whT"][k * 128:(k + 1) * 128, :])
        nc.sync.dma_start(w1T_s[:, k], io["w1T"][k * 128:(k + 1) * 128, :])
    wmv_s = wpool.tile([128, KC, 3], BF16)
    for k in range(KC):
        nc.sync.dma_start(wmv_s[:, k], io["wmv"][k * 128:(k + 1) * 128, :])
    bh1_s = wpool.tile([128, KC], F32)
    b1_s = wpool.tile([128, KC], F32)
    wt_s = wpool.tile([128, KC], F32)
    nwt_s = wpool.tile([128, KC], F32)
    nc.sync.dma_start(bh1_s[:], io["bh1c"][:])
    nc.sync.dma_start(b1_s[:], io["b1c"][:])
    nc.sync.dma_start(wt_s[:], io["wtc"][:])
    nc.sync.dma_start(nwt_s[:], io["nwtc"][:])
    br_s = wpool.tile([1, 1], F32)
    nc.sync.dma_start(br_s[:], io["br"][:])
    ones_s = wpool.tile([1, 128], F32)
    nc.vector.memset(ones_s[:], 1.0)

    # working pools
    sb = ctx.enter_context(tc.tile_pool(name="sb", bufs=2))
    ps = ctx.enter_context(tc.tile_pool(name="ps", bufs=2, space="PSUM"))

    for i in range(NTILE):
        # ---- t tile + broadcast across partitions (ones-matmul) ----
        t_s = sb.tile([1, T], F32, tag="t_s", bufs=2)
        nc.sync.dma_start(t_s[:], td[i:i + 1, :])
        ptb = ps.tile([128, T], F32, tag="pmm", bufs=3)
        nc.tensor.matmul(ptb[:], lhsT=ones_s[:], rhs=t_s[:], start=True, stop=True)
        tB = sb.tile([128, T], F32, tag="tB", bufs=2)
        nc.scalar.copy(tB[:], ptb[:])

        # ---- X = h^T chunks via DMA transpose ----
        Xs = []
        for k in range(KC):
            X = sb.tile([128, T], BF16, tag="X", bufs=6)
            nc.sync.dma_start(
                X[:], hb[i * T:(i + 1) * T, k * 128:(k + 1) * 128], transpose=True
            )
            Xs.append(X)

        # ---- layer 1: h2 chunks + elementwise (a0, a, da) ----
        a0s, as_, das = [], [], []
        for m in range(KC):
            pv1 = ps.tile([128, T], F32, tag="pv1", bufs=2)
            for k in range(KC):
                nc.tensor.matmul(
                    pv1[:],
                    lhsT=whT_s[:, k, m * 128:(m + 1) * 128],
                    rhs=Xs[k][:],
                    start=(k == 0),
                    stop=(k == KC - 1),
                )
            a0 = sb.tile([128, T], BF16, tag="a0", bufs=6)
            nc.scalar.activation(a0[:], pv1[:], AF.Tanh, bias=bh1_s[:, m:m + 1])
            u = sb.tile([128, T], F32, tag="u", bufs=2)
            nc.vector.scalar_tensor_tensor(
                u[:], in0=tB[:], scalar=wt_s[:, m:m + 1], in1=pv1[:],
                op0=ALU.mult, op1=ALU.add,
            )
            a = sb.tile([128, T], BF16, tag="a", bufs=6)
            nc.scalar.activation(a[:], u[:], AF.Tanh, bias=bh1_s[:, m:m + 1])
            sq = sb.tile([128, T], BF16, tag="sq", bufs=2)
            nc.vector.tensor_mul(sq[:], a[:], a[:])
            da = sb.tile([128, T], BF16, tag="da", bufs=6)
            # da = wt - wt*a^2  (= wt*(1-a^2))
            nc.vector.tensor_scalar(
                da[:], sq[:], nwt_s[:, m:m + 1], wt_s[:, m:m + 1],
                op0=ALU.mult, op1=ALU.add,
            )
            a0s.append(a0)
            as_.append(a)
            das.append(da)

        # ---- res = relu(W_r·h + b_r) ----
        pres = ps.tile([1, T], F32, tag="pres", bufs=1)
        for k in range(KC):
            nc.tensor.matmul(
                pres[:], lhsT=wmv_s[:, k, 0:1], rhs=Xs[k][:],
                start=(k == 0), stop=(k == KC - 1),
            )
        res = sb.tile([1, T], F32, tag="res", bufs=2)
        nc.scalar.activation(res[:], pres[:], AF.Relu, bias=br_s[0:1, 0:1])

        # ---- layer 2 (z0, z, g) + gradient elementwise ----
        dzzs, dznegs = [], []
        for m in range(KC):
            pv0 = ps.tile([128, T], F32, tag="pmm", bufs=3)
            for k in range(KC):
                nc.tensor.matmul(
                    pv0[:], lhsT=w1T_s[:, k, m * 128:(m + 1) * 128], rhs=a0s[k][:],
                    start=(k == 0), stop=(k == KC - 1),
                )
            z0 = sb.tile([128, T], BF16, tag="z0", bufs=3)
            nc.scalar.activation(z0[:], pv0[:], AF.Tanh, bias=b1_s[:, m:m + 1])

            pv = ps.tile([128, T], F32, tag="pmm", bufs=3)
            for k in range(KC):
                nc.tensor.matmul(
                    pv[:], lhsT=w1T_s[:, k, m * 128:(m + 1) * 128], rhs=as_[k][:],
                    start=(k == 0), stop=(k == KC - 1),
                )
            z = sb.tile([128, T], BF16, tag="z", bufs=3)
            nc.scalar.activation(z[:], pv[:], AF.Tanh, bias=b1_s[:, m:m + 1])

            pg = ps.tile([128, T], F32, tag="pmm", bufs=3)
            for k in range(KC):
                nc.tensor.matmul(
                    pg[:], lhsT=w1T_s[:, k, m * 128:(m + 1) * 128], rhs=das[k][:],
                    start=(k == 0), stop=(k == KC - 1),
                )
            zq = sb.tile([128, T], BF16, tag="zq", bufs=2)
            nc.vector.tensor_mul(zq[:], z[:], z[:])
            dzneg = sb.tile([128, T], BF16, tag="dzneg", bufs=3)
            # (z^2 - 1) * g   (negated dz; folded into -W2p in the matvec)
            nc.vector.scalar_tensor_tensor(
                dzneg[:], in0=zq[:], scalar=1.0, in1=pg[:],
                op0=ALU.subtract, op1=ALU.mult,
            )
            dzz = sb.tile([128, T], BF16, tag="dzz", bufs=3)
            nc.vector.tensor_sub(dzz[:], z[:], z0[:])
            dzzs.append(dzz)
            dznegs.append(dzneg)

        # ---- output matvecs + epilogue ----
        phaz = ps.tile([1, T], F32, tag="phaz", bufs=1)
        for k in range(KC):
            nc.tensor.matmul(
                phaz[:], lhsT=wmv_s[:, k, 1:2], rhs=dzzs[k][:],
                start=(k == 0), stop=(k == KC - 1),
            )
        pint = ps.tile([1, T], F32, tag="pint", bufs=1)
        for k in range(KC):
            nc.tensor.matmul(
                pint[:], lhsT=wmv_s[:, k, 2:3], rhs=dznegs[k][:],
                start=(k == 0), stop=(k == KC - 1),
            )

        rt = sb.tile([1, T], F32, tag="rt", bufs=2)
        nc.vector.tensor_mul(rt[:], res[:], t_s[:])
        hz = sb.tile([1, T], F32, tag="hz", bufs=2)
        nc.vector.tensor_add(hz[:], rt[:], phaz[:])
        it = sb.tile([1, T], F32, tag="it", bufs=2)
        nc.vector.scalar_tensor_tensor(
            it[:], in0=res[:], scalar=MINPOS, in1=pint[:],
            op0=ALU.add, op1=ALU.add,
        )
        nc.sync.dma_start(out_h[i:i + 1, :], hz[:])
        nc.sync.dma_start(out_i[i:i + 1, :], it[:])


def build_nc():
    nc = bacc.Bacc(
        "TRN2", target_bir_lowering=False, debug=False, enable_asserts=False
    )
    io = {
        "hb": nc.dram_tensor("hb", [NTOK, H], BF16, kind="ExternalInput").ap(),
        "t": nc.dram_tensor("t", [NTILE, T], F32, kind="ExternalInput").ap(),
        "whT": nc.dram_tensor("whT", [H, H], BF16, kind="ExternalInput").ap(),
        "w1T": nc.dram_tensor("w1T", [H, H], BF16, kind="ExternalInput").ap(),
        "wmv": nc.dram_tensor("wmv", [H, 3], BF16, kind="ExternalInput").ap(),
        "bh1c": nc.dram_tensor("bh1c", [128, KC], F32, kind="ExternalInput").ap(),
        "b1c": nc.dram_tensor("b1c", [128, KC], F32, kind="ExternalInput").ap(),
        "wtc": nc.dram_tensor("wtc", [128, KC], F32, kind="ExternalInput").ap(),
        "nwtc": nc.dram_tensor("nwtc", [128, KC], F32, kind="ExternalInput").ap(),
        "br": nc.dram_tensor("br", [1, 1], F32, kind="ExternalInput").ap(),
        "intensity": nc.dram_tensor(
            "intensity", [NTILE, T], F32, kind="ExternalOutput"
        ).ap(),
        "hazard": nc.dram_tensor(
            "hazard", [NTILE, T], F32, kind="ExternalOutput"
        ).ap(),
    }
    with tile.TileContext(nc) as tc:
        with ExitStack() as ctx:
            _build_body(ctx, tc, io)
    nc.compile()
    return nc


def _get_nc():
    global _NC_CACHE
    if _NC_CACHE is None:
        _NC_CACHE = build_nc()
    return _NC_CACHE


def prep_in_maps(inputs: dict) -> list[dict]:
    """Host-side preprocessing: relu-mask params, transpose/cast, shard."""
    bf = ml_dtypes.bfloat16
    f32 = np.float32

    def arr(name):
        return np.asarray(inputs[name], f32)

    h, t = arr("h"), arr("t")
    relu = lambda x: np.maximum(x, 0.0)
    wt = relu(arr("w_t1"))
    W1p = relu(arr("W1"))
    b1p = relu(arr("b1"))
    W2p = relu(arr("W2"))
    W_r, b_r, b_h1, W_h1 = arr("W_r"), arr("b_r"), arr("b_h1"), arr("W_h1")

    whT = np.ascontiguousarray(W_h1.T).astype(bf)
    w1T = np.ascontiguousarray(W1p.T).astype(bf)
    wmv = np.ascontiguousarray(
        np.stack([W_r[0], W2p[0], -W2p[0]], axis=1)
    ).astype(bf)

    def chunked(v):  # [512] -> [128, KC] with [p, c] = v[c*128+p]
        return np.ascontiguousarray(v.reshape(KC, 128).T).astype(f32)

    shared = {
        "whT": whT,
        "w1T": w1T,
        "wmv": wmv,
        "bh1c": chunked(b_h1),
        "b1c": chunked(b1p),
        "wtc": chunked(wt),
        "nwtc": chunked(-wt),
        "br": np.array([[b_r[0]]], f32),
    }
    hb_all = h.reshape(NCORES, NTOK, H).astype(bf)
    t_all = t.reshape(NCORES, NTILE, T).astype(f32)
    return [
        {
            "hb": np.ascontiguousarray(hb_all[c]),
            "t": np.ascontiguousarray(t_all[c]),
            **shared,
        }
        for c in range(NCORES)
    ]


def run(inputs: dict, trace: bool = False):
    nc = _get_nc()
    in_maps = prep_in_maps(inputs)
    res = bass_utils.run_bass_kernel_spmd(
        nc, in_maps, core_ids=list(range(NCORES)), trace=trace
    )
    intensity = (
        np.concatenate([r["intensity"].reshape(-1) for r in res.results])
        .reshape(B, S)
        .astype(np.float32)
    )
    hazard = (
        np.concatenate([r["hazard"].reshape(-1) for r in res.results])
        .reshape(B, S, 1)
        .astype(np.float32)
    )
    return (intensity, hazard), res


def kernel(**inputs):
    (intensity, hazard), _ = run(inputs)
    return intensity, hazard
